# revision 11
# baseline (speedup 1.0000x reference)
"""Trainium2 Bass kernel for nn_MicroTransformerLayer — v2.

Sharding: 8 cores = 4 sequences x 2 interleaved (even/odd token) shards.
Each core's view permutes its sequence so the OTHER parity's tokens occupy
context positions 0:1024 (global order) and its OWN parity's tokens occupy
1024:2048. Causality in view space becomes: own-vs-own = plain tril in rank
space; own-vs-other = tril shifted by jj (0 or 1) — supplied as host-side
mask tensors, so the program stays SPMD-uniform while each core does an
equal, near-minimal share of causal attention work.

Everything is stored bf16 (fp32 PSUM accumulation); rel-err budget 2e-2
makes this safe. One activation table (ln/exp) serves rmsnorm
(rsqrt = exp(-0.5 ln)), softmax exp, and silu (via exp).
"""

import os
import sys

for _p in ("/opt/trn_rl_repo", "/root/.axon_site/_ro/trn_rl_repo"):
    if os.path.isdir(_p) and _p not in sys.path:
        sys.path.append(_p)

import numpy as np
from ml_dtypes import bfloat16

import concourse.bass as bass
import concourse.mybir as mybir
import concourse.tile as tile
from concourse import bacc
from concourse.bass_utils import run_bass_kernel_spmd

F32 = mybir.dt.float32
BF16 = mybir.dt.bfloat16
AF = mybir.ActivationFunctionType
MUL = mybir.AluOpType.mult
DIV = mybir.AluOpType.divide

BIG, SMALL, HEADS, HD, FF = 4096, 256, 4, 64, 512
B, T = 4, 2048
SEQ, OWN = 2048, 1024
P, CH, QC = 128, 512, 256
N_CTX_CH = SEQ // CH          # 4
KT_BIG = BIG // P             # 32
EPS = 1.1920929e-07
N_CORES = 8


def _emit(nc, tc, d):
    ts, ds = bass.ts, bass.ds
    with (
        nc.allow_low_precision(reason="bf16 kernel; rel-err budget 2e-2"),
        tc.tile_pool(name="persist", bufs=1) as pp,
        tc.tile_pool(name="xin", bufs=4) as xin,
        tc.tile_pool(name="hnp", bufs=2) as hnp,
        tc.tile_pool(name="prp", bufs=3) as prp,
        tc.tile_pool(name="work", bufs=1) as wk,
        tc.tile_pool(name="wk2", bufs=3) as wk2,
        tc.tile_pool(name="outp", bufs=6) as outp,
        tc.tile_pool(name="psA", bufs=2, space="PSUM") as psA,
        tc.tile_pool(name="psS", bufs=2, space="PSUM") as psS,
        tc.tile_pool(name="psO", bufs=2, space="PSUM") as psO,
    ):
        # ---------------- persistent SBUF tensors ----------------
        ones_s = pp.tile([P, P], BF16, tag="ones")
        eps_s = pp.tile([P, 2], F32, tag="eps")
        w_qkv = pp.tile([P, 2, 3 * SMALL], BF16, tag="wqkv")
        w_o = pp.tile([P, 2, SMALL], BF16, tag="wo")
        masks = pp.tile([P, 8, 2 * CH], BF16, tag="masks")

        def _late_small_loads():
            # issued after the first x/W_down tiles so they don't delay start
            nc.sync.dma_start(eps_s[:], d["epsb"])
            nc.sync.dma_start(ones_s[:], d["ones"])
            nc.sync.dma_start(
                w_qkv[:], d["wqkv"].rearrange("(ko ki) m -> ki ko m", ki=P))
            nc.sync.dma_start(
                w_o[:], d["wo"].rearrange("(ko ki) m -> ki ko m", ki=P))

        w_dd = pp.tile([P, KT_BIG, SMALL], BF16, tag="wdd")
        w_up = pp.tile([P, 2, BIG], BF16, tag="wup")
        w_gu = pp.tile([P, 2, 2 * FF], BF16, tag="wgu")
        w_dff = pp.tile([P, 4, SMALL], BF16, tag="wdff")

        riT = pp.tile([P, 16], F32, tag="riT")     # 1/rms per token (col/view-tile)
        riTs = pp.tile([P, 16], F32, tag="riTs")   # 0.125/rms, softmax exp scale
        hT = pp.tile([P, 2, OWN], BF16, tag="hT")
        kT = pp.tile([P, 2, SEQ], BF16, tag="kT")
        qT = pp.tile([P, 2, OWN], BF16, tag="qT")
        vo = pp.tile([P, 16, HEADS * (HD + 1)], BF16, tag="vo")
        aoT = pp.tile([P, 2, OWN], BF16, tag="aoT")

        def _vones_load():
            # ones column of each head's V block
            nc.sync.dma_start(
                vo[:].rearrange("p t (h x) -> p t h x", x=HD + 1)[:, :, :, HD : HD + 1],
                d["vones"].rearrange("p (t h) -> p t h", h=HEADS)[:, :, :, None],
            )

        # =============== stage A: down-proj + norm1 + QKV (one view chunk) ==
        # rmsnorm is applied LATE: K,Q projections run on unnormalized h and
        # the per-token 1/rms folds into the PSUM->SBUF copy (elementwise for
        # feature-major K/Q, per-partition scalar for token-major V). The PE
        # therefore never waits on the norm chain.
        def emit_A(c, after_first_dma=None):
            cs = ds(c * CH, CH)
            ph = [psA.tile([P, CH], F32, tag="dn", name=f"ph{m}") for m in range(2)]
            grp = 4 if c == 0 else 8   # finer granularity to start PE sooner
            for kb in range(KT_BIG // grp):
                if c == 0 and kb < 2:
                    nc.sync.dma_start(
                        w_dd[:, ts(kb, 4), :],
                        d["wd"].rearrange("(ko ki) m -> ki ko m", ki=P)[:, ts(kb, 4), :],
                    )
                elif c == 0 and kb % 2 == 0:
                    q = kb // 2
                    if q >= 1:
                        nc.sync.dma_start(
                            w_dd[:, ts(q, 8), :],
                            d["wd"].rearrange("(ko ki) m -> ki ko m", ki=P)[:, ts(q, 8), :],
                        )
                xt = xin.tile([P, grp, CH], BF16, tag="xt")
                nc.sync.dma_start(
                    xt[:],
                    d["xT"].rearrange("(ko ki) t -> ki ko t", ki=P)[:, ts(kb, grp), cs],
                )
                if kb == 0 and after_first_dma is not None:
                    after_first_dma()
                for kk in range(grp):
                    k = grp * kb + kk
                    for m in range(2):
                        nc.tensor.matmul(
                            ph[m][:], w_dd[:, k, ts(m, P)], xt[:, kk, :],
                            start=(k == 0), stop=(k == KT_BIG - 1),
                        )
            hch = hnp.tile([P, 2, CH], BF16, tag="hch")
            hdst = (lambda m: hT[:, m, ds((c - 2) * CH, CH)]) if c >= 2 else (
                lambda m: hch[:, m, :])
            hsq = [wk2.tile([P, CH], BF16, tag="hsq", name=f"hsq{m}") for m in range(2)]
            nc.vector.tensor_copy(hdst(0), ph[0][:])
            nc.scalar.copy(hdst(1), ph[1][:])
            nc.vector.tensor_mul(hsq[0][:], hdst(0), hdst(0))
            nc.scalar.activation(hsq[1][:], ph[1][:], AF.Square)
            # feature-major sumsq (only the Q copies need it) ...
            pss = None
            if c >= 2:
                pss = psA.tile([P, CH], F32, tag="dn")
                for m in range(2):
                    nc.tensor.matmul(pss[:], ones_s[:], hsq[m][:],
                                     start=(m == 0), stop=(m == 1))
            # ... and token-major sumsq (for V scaling); one accumulation
            # group spans all 4 columns (PSUM zero regions are 2KB-wide)
            pst = psA.tile([P, 4], F32, tag="dn")
            for tt in range(4):
                for m in range(2):
                    nc.tensor.matmul(pst[:, tt : tt + 1],
                                     hsq[m][:, ts(tt, P)], ones_s[:, 0:1],
                                     start=(tt == 0 and m == 0),
                                     stop=(tt == 3 and m == 1))
            # K~ (stored unnormalized; 1/rms_k folds into the softmax exp's
            # per-partition scale) and Q~ (1/rms_q folds into its copy)
            pk = [psA.tile([P, CH], F32, tag="dn", name=f"pk{m}") for m in range(2)]
            for m in range(2):
                for kt in range(2):
                    nc.tensor.matmul(pk[m][:], w_qkv[:, kt, ds(SMALL + m * P, P)],
                                     hdst(kt), start=(kt == 0), stop=(kt == 1))
            nc.vector.tensor_copy(kT[:, 0, cs], pk[0][:])
            nc.scalar.copy(kT[:, 1, cs], pk[1][:])
            pq = None
            if c >= 2:
                pq = [psA.tile([P, CH], F32, tag="dn", name=f"pq{m}") for m in range(2)]
                for m in range(2):
                    for kt in range(2):
                        nc.tensor.matmul(pq[m][:], w_qkv[:, kt, ds(m * P, P)],
                                         hdst(kt), start=(kt == 0), stop=(kt == 1))
            # V~ token-major from unnormalized h
            pv = [psA.tile([P, SMALL], F32, tag="dn", name=f"pv{tt}")
                  for tt in range(4)]
            for tt in range(4):
                for kt in range(2):
                    nc.tensor.matmul(pv[tt][:], hdst(kt)[:, ts(tt, P)],
                                     w_qkv[:, kt, ds(2 * SMALL, SMALL)],
                                     start=(kt == 0), stop=(kt == 1))
            # norm chain (ACT), off the PE critical path
            rinv = None
            if c >= 2:  # feature-major 1/rms only feeds the Q copies
                lnv = wk.tile([P, CH], F32, tag="lnv")
                nc.scalar.activation(lnv[:], pss[:], AF.Ln, scale=1.0 / SMALL,
                                     bias=eps_s[:, 0:1])
                rinv = wk.tile([P, CH], F32, tag="rinv")
                nc.scalar.activation(rinv[:], lnv[:], AF.Exp, scale=-0.5)
            lnt = wk.tile([P, 4], F32, tag="lnt")
            nc.scalar.activation(lnt[:], pst[:], AF.Ln, scale=1.0 / SMALL,
                                 bias=eps_s[:, 0:1])
            nc.scalar.activation(riT[:, ts(c, 4)], lnt[:], AF.Exp, scale=-0.5)
            # 0.125/rms_k = exp(-0.5 ln v + ln(1/8)), the softmax exp scale
            nc.scalar.activation(riTs[:, ts(c, 4)], lnt[:], AF.Exp, scale=-0.5,
                                 bias=eps_s[:, 1:2])
            # scaled copies to Q/V stores
            if pq is not None:
                for m in range(2):
                    nc.vector.tensor_mul(qT[:, m, ds((c - 2) * CH, CH)],
                                         pq[m][:], rinv[:])
            for tt in range(4):
                ct = 4 * c + tt
                dst = vo[:, ct, :].rearrange("p (h x) -> p h x", x=HD + 1)[:, :, 0:HD]
                src = pv[tt][:].rearrange("p (h x) -> p h x", x=HD)
                if tt % 2 == 0:
                    nc.vector.tensor_scalar_mul(dst, src, riT[:, 4 * c + tt : 4 * c + tt + 1])
                else:
                    nc.scalar.mul(dst, src, riT[:, 4 * c + tt : 4 * c + tt + 1])

        # =============== stage B: attention for one own q-chunk of 512 ======
        # head pair ft = (2ft, 2ft+1); each head's scores/AV slice spans a
        # full 512-f32 PSUM bank (accumulation zero regions are bank-wide).
        def emit_B(c2, ft):
            qs = ds(c2 * CH, CH)
            vis = [t for t in range(4 * c2 + 4)] + [8 + t for t in range(4 * c2 + 4)]
            band = {4 * c2 + i: 4 + i for i in range(4)}
            band.update({8 + 4 * c2 + i: i for i in range(4)})
            po = [psO.tile([P, CH], F32, tag="po", name=f"po{hh}")
                  for hh in range(2)]
            for i, kt in enumerate(vis):
                pss = psS.tile([P, 2 * CH], F32, tag="sc")
                for hh in range(2):
                    b0 = HD * hh
                    nc.tensor.matmul(
                        pss[:, ts(hh, CH)],
                        kT[b0 : b0 + HD, ft, ts(kt, P)],
                        qT[b0 : b0 + HD, ft, qs],
                        start=True, stop=True,
                    )
                pr = prp.tile([P, 2 * CH], BF16, tag="pr")
                nc.scalar.activation(pr[:], pss[:], AF.Exp,
                                     scale=riTs[:, kt : kt + 1])
                if kt in band:
                    nc.vector.tensor_mul(pr[:], pr[:], masks[:, band[kt], :])
                for hh in range(2):
                    h = 2 * ft + hh
                    nc.tensor.matmul(
                        po[hh][0 : HD + 1, :], vo[:, kt, ts(h, HD + 1)],
                        pr[:, ts(hh, CH)],
                        start=(i == 0), stop=(i == len(vis) - 1),
                    )
            # copy raw accumulators to SBUF first (frees the po banks for
            # the next pass), then normalize off-PSUM: broadcast row 64,
            # reciprocal, scale.
            pos = wk.tile([P, 2, CH], BF16, tag="pos")
            rb = wk.tile([P, 2 * CH], BF16, tag="rb")
            for hh in range(2):
                if hh == 0:
                    nc.vector.tensor_copy(pos[0 : HD + 1, hh, :],
                                          po[hh][0 : HD + 1, :])
                else:
                    nc.scalar.copy(pos[0 : HD + 1, hh, :],
                                   po[hh][0 : HD + 1, :])
                pb = psA.tile([P, CH], F32, tag="dn")
                nc.tensor.matmul(pb[:], ones_s[HD : HD + 1, 0:P],
                                 pos[HD : HD + 1, hh, :],
                                 start=True, stop=True)
                nc.vector.reciprocal(rb[0:HD, ts(hh, CH)], pb[0:HD, :])
                nc.vector.tensor_mul(
                    aoT[HD * hh : HD * hh + HD, ft, qs],
                    pos[0:HD, hh, :], rb[0:HD, ts(hh, CH)],
                )

        # =============== stage C part 1: o-proj + norm2 + FF ================
        def emit_C_head(cc):
            qs = ds(cc * CH, CH)
            h2 = wk.tile([P, 2, CH], F32, tag="h2", name=f"h2_{cc}")
            for m in range(2):
                pp_ = psA.tile([P, CH], F32, tag="dn")
                for kt in range(2):
                    nc.tensor.matmul(pp_[:], w_o[:, kt, ts(m, P)],
                                     aoT[:, kt, qs], start=(kt == 0), stop=(kt == 1))
                nc.vector.tensor_add(h2[:, m, :], pp_[:], hT[:, m, qs])
            h2sq = [wk2.tile([P, CH], BF16, tag="hsq", name=f"h2sq{m}") for m in range(2)]
            for m in range(2):
                nc.scalar.activation(h2sq[m][:], h2[:, m, :], AF.Square)
            pss = psA.tile([P, CH], F32, tag="dn")
            for m in range(2):
                nc.tensor.matmul(pss[:], ones_s[:], h2sq[m][:],
                                 start=(m == 0), stop=(m == 1))
            lnv = wk.tile([P, CH], F32, tag="lnv")
            nc.scalar.activation(lnv[:], pss[:], AF.Ln, scale=1.0 / SMALL,
                                 bias=eps_s[:, 0:1])
            rinv = wk.tile([P, CH], F32, tag="rinv")
            nc.scalar.activation(rinv[:], lnv[:], AF.Exp, scale=-0.5)
            hn2 = wk.tile([P, 2, CH], BF16, tag="hn2", name=f"hn2_{cc}")
            for m in range(2):
                nc.vector.tensor_mul(hn2[:, m, :], h2[:, m, :], rinv[:])
            # FF: silu(g)*u = (g*u) / (1 + exp(-g))
            fT = wk.tile([P, 4, CH], BF16, tag="fT", name=f"fT_{cc}")
            for g in range(4):
                pgate = psA.tile([P, CH], F32, tag="dn")
                for kt in range(2):
                    nc.tensor.matmul(pgate[:], w_gu[:, kt, ts(g, P)],
                                     hn2[:, kt, :], start=(kt == 0), stop=(kt == 1))
                pup = psA.tile([P, CH], F32, tag="dn")
                for kt in range(2):
                    nc.tensor.matmul(pup[:], w_gu[:, kt, ds(FF + g * P, P)],
                                     hn2[:, kt, :], start=(kt == 0), stop=(kt == 1))
                # stage gate/up to SBUF right away so the PSUM bank frees
                # for the next g's matmuls; silu chain then runs off-SBUF
                pg_s = wk2.tile([P, CH], BF16, tag="pgs")
                nc.vector.tensor_copy(pg_s[:], pgate[:])
                pu_s = wk2.tile([P, CH], BF16, tag="pus")
                nc.scalar.copy(pu_s[:], pup[:])
                ex = wk2.tile([P, CH], BF16, tag="ex")
                nc.scalar.activation(ex[:], pg_s[:], AF.Exp, scale=-1.0)
                ex1 = wk2.tile([P, CH], BF16, tag="ex1")
                nc.scalar.add(ex1[:], ex[:], 1.0)
                rc = wk2.tile([P, CH], BF16, tag="rc")
                nc.vector.reciprocal(rc[:], ex1[:])
                xs = wk2.tile([P, CH], BF16, tag="xs")
                nc.vector.tensor_mul(xs[:], pg_s[:], rc[:])
                nc.vector.tensor_mul(fT[:, g, :], xs[:], pu_s[:])
            return h2, fT

        def emit_C_down(cc, h2, fT):
            # FF down-proj + residual (split off so other PE work can sit
            # between the silu chain and its consumer)
            h3 = wk.tile([P, 2, CH], BF16, tag="h3", name=f"h3_{cc}")
            for m in range(2):
                pf = psA.tile([P, CH], F32, tag="dn")
                for kt in range(4):
                    nc.tensor.matmul(pf[:], w_dff[:, kt, ts(m, P)],
                                     fT[:, kt, :], start=(kt == 0), stop=(kt == 3))
                nc.vector.tensor_add(h3[:, m, :], pf[:], h2[:, m, :])
            return h3

        # =============== stage C part 2: up-projection ======================
        def emit_C_up(cc, h3, mbs=None):
            qs = ds(cc * CH, CH)
            for mb in (mbs if mbs is not None else range(KT_BIG // 4)):
                last = (cc == 1 and mb == KT_BIG // 4 - 1)
                yt = outp.tile([P, 4, CH], BF16, tag="yt")
                ydst = d["yT"].rearrange("(mo ki) t -> ki mo t", ki=P)[:, ts(mb, 4), qs]
                for kk in range(4):
                    m = 4 * mb + kk
                    py = psA.tile([P, CH], F32, tag="dn")
                    for kt in range(2):
                        nc.tensor.matmul(py[:], w_up[:, kt, ts(m, P)],
                                         h3[:, kt, :], start=(kt == 0), stop=(kt == 1))
                    if (m % 2 == 0) or last:
                        nc.vector.tensor_copy(yt[:, kk, :], py[:])
                    else:
                        nc.scalar.copy(yt[:, kk, :], py[:])
                    if last and kk == 1:
                        # drain the first half early to shorten the tail
                        nc.sync.dma_start(ydst[:, 0:2, :], yt[:, 0:2, :])
                if last:
                    nc.scalar.dma_start(ydst[:, 2:4, :], yt[:, 2:4, :])
                else:
                    eng = nc.scalar if mb % 2 == 0 else nc.sync
                    eng.dma_start(ydst, yt[:])

        # ---------------- interleaved emission ----------------
        emit_A(0, after_first_dma=_late_small_loads)
        emit_A(1, after_first_dma=_vones_load)
        emit_A(2)
        nc.sync.dma_start(masks[:], d["masks"])
        emit_B(0, 0)
        emit_B(0, 1)
        nc.sync.dma_start(w_gu[:], d["wgu"].rearrange("(ko ki) m -> ki ko m", ki=P))
        nc.sync.dma_start(w_dff[:], d["wdff"].rearrange("(ko ki) m -> ki ko m", ki=P))
        emit_A(3)
        for q in range(4):
            nc.sync.dma_start(
                w_up[:, :, ts(q, BIG // 4)],
                d["wup"].rearrange("(ko ki) m -> ki ko m", ki=P)[:, :, ts(q, BIG // 4)],
            )
        emit_B(1, 0)
        h2_0, fT_0 = emit_C_head(0)
        emit_B(1, 1)
        h3_0 = emit_C_down(0, h2_0, fT_0)
        emit_C_up(0, h3_0, mbs=range(0, 2))
        h2_1, fT_1 = emit_C_head(1)
        emit_C_up(0, h3_0, mbs=range(2, 8))
        h3_1 = emit_C_down(1, h2_1, fT_1)
        emit_C_up(1, h3_1)


class _Bacc(bacc.Bacc):
    """Bacc whose act-table placement is steered to a single table.

    Every activation this kernel emits (ln, exp, square, copy, identity)
    lives in `natural_log_exp_and_others`; the stock greedy placement
    alternates between per-function first-match tables, inserting a
    1.3us table load per rmsnorm/softmax transition. Pruning those
    functions from every other table in the pass's analysis input makes
    it settle on the one table that serves them all (the emitted
    act_func_set_id still indexes the unmodified act_info.json order, so
    runtime behaviour is exact).
    """

    _ONE_TABLE_FUNCS = None  # set of AF values, filled at first use

    def insert_act_table_loads(self):
        import bass_rust as _bass_rust
        from concourse.hw_specs import get_activation_tables

        has_activation = any(
            isinstance(i, mybir.InstActivation)
            for b in self.main_func.blocks
            for i in b.instructions
        )
        if not has_activation:
            return
        mine = {AF.Ln, AF.Exp, AF.Square, AF.Copy, AF.Identity}
        keep = "natural_log_exp_and_others"
        tables = []
        for name, s in get_activation_tables(self.m.arch).items():
            s = set(s) if name == keep else set(s) - mine
            tables.append((name, s))
        _bass_rust.insert_act_table_loads(self, tables)


def _build():
    nc = _Bacc("TRN2", target_bir_lowering=False, debug=False,
               num_devices=N_CORES)
    d = {}
    d["xT"] = nc.dram_tensor("xT", [BIG, SEQ], BF16, kind="ExternalInput").ap()
    d["wd"] = nc.dram_tensor("wd", [BIG, SMALL], BF16, kind="ExternalInput").ap()
    d["wqkv"] = nc.dram_tensor("wqkv", [SMALL, 3 * SMALL], BF16, kind="ExternalInput").ap()
    d["wo"] = nc.dram_tensor("wo", [SMALL, SMALL], BF16, kind="ExternalInput").ap()
    d["wgu"] = nc.dram_tensor("wgu", [SMALL, 2 * FF], BF16, kind="ExternalInput").ap()
    d["wdff"] = nc.dram_tensor("wdff", [FF, SMALL], BF16, kind="ExternalInput").ap()
    d["wup"] = nc.dram_tensor("wup", [SMALL, BIG], BF16, kind="ExternalInput").ap()
    d["ones"] = nc.dram_tensor("ones", [P, P], BF16, kind="ExternalInput").ap()
    d["masks"] = nc.dram_tensor("masks", [P, 8, 2 * CH], BF16, kind="ExternalInput").ap()
    d["epsb"] = nc.dram_tensor("epsb", [P, 2], F32, kind="ExternalInput").ap()
    d["vones"] = nc.dram_tensor("vones", [P, 64], BF16, kind="ExternalInput").ap()
    d["yT"] = nc.dram_tensor("yT", [BIG, OWN], BF16, kind="ExternalOutput").ap()
    with tile.TileContext(nc) as tc:
        _emit(nc, tc, d)
    nc.compile()
    return nc


_NC_CACHE = None


def _get_nc():
    global _NC_CACHE
    if _NC_CACHE is None:
        _NC_CACHE = _build()
    return _NC_CACHE


def make_in_maps(x, W_down, W_up, W_qkv, W_o, W_gate, W_upff, W_downff, g1, g2):
    bf = bfloat16
    shared = {
        "wd": np.ascontiguousarray(W_down.T).astype(bf),
        "wqkv": np.ascontiguousarray((np.asarray(W_qkv) * np.asarray(g1)[None, :]).T).astype(bf),
        "wo": np.ascontiguousarray(W_o.T).astype(bf),
        "wgu": np.ascontiguousarray(
            (np.concatenate([W_gate, W_upff], axis=0) * np.asarray(g2)[None, :]).T
        ).astype(bf),
        "wdff": np.ascontiguousarray(W_downff.T).astype(bf),
        "wup": np.ascontiguousarray(W_up.T).astype(bf),
        "ones": np.ones((P, P), bf),
        "vones": np.ones((P, 64), bf),
        "epsb": np.stack([np.full(P, EPS, np.float32),
                          np.full(P, np.log(0.125), np.float32)], axis=1),
    }
    kk = np.arange(P)[:, None]
    qq = np.arange(CH)[None, :]
    x = np.asarray(x)
    in_maps = []
    for b in range(B):
        for jj in range(2):
            other = x[b, (1 - jj)::2]           # [1024, BIG] global order
            own = x[b, jj::2]
            xp = np.concatenate([other, own], axis=0)       # [SEQ, BIG]
            m = dict(shared)
            m["xT"] = np.ascontiguousarray(xp.T).astype(bf)  # [BIG, SEQ]
            ms = [(kk <= qq - 128 * i) for i in range(4)]
            mo = [(kk <= qq - 128 * i - 1 + jj) for i in range(4)]
            mask = np.stack([np.tile(mm.astype(bf), (1, 2))
                             for mm in (ms + mo)], axis=1)
            m["masks"] = np.ascontiguousarray(mask)          # [P, 8, 1024]
            in_maps.append(m)
    return in_maps


def assemble(results):
    y = np.empty((B, T, BIG), np.float32)
    for b in range(B):
        for jj in range(2):
            yT = results[2 * b + jj]["yT"]                   # [BIG, OWN] bf16
            y[b, jj::2] = yT.T.astype(np.float32)
    return y


def kernel(x, W_down, W_up, W_qkv, W_o, W_gate, W_upff, W_downff, g1, g2):
    nc = _get_nc()
    in_maps = make_in_maps(x, W_down, W_up, W_qkv, W_o, W_gate, W_upff,
                           W_downff, g1, g2)
    res = run_bass_kernel_spmd(nc, in_maps, core_ids=list(range(N_CORES)))
    return assemble(res.results)
